# revision 21
# baseline (speedup 1.0000x reference)
"""Multi-level RoIAlign (FPN BaseRoIHead) as a Trainium2 Bass kernel.

Contract: kernel(**inputs) takes the FULL unsharded inputs
(p2..p6: [2,H,W,256] f32, proposals: [2,512,4] f32) and returns the FULL
output [2, 512, 7, 7, 256] f32.

Strategy:
- Shard the 2*512 RoIs over 8 cores (128 RoIs each; cores 0-3 image 0,
  cores 4-7 image 1). Each core receives its image's feature pyramid in a
  "row-pair" layout: featp[base_l + y*W + x] = concat(feat[y,x,:], feat[y+1,x,:])
  so that the 4 bilinear corners of one sample point are ONE contiguous 4KB
  block (positions (y0,x0) and (y0,x0+1)).
- On device: per-RoI level assignment via exact area thresholds, sample
  coordinates / bilinear weights / gather indices computed on DVE in a
  [128 RoIs x 196 sample points] layout; 196 indirect DMA gathers of
  [128, 1024] f32 (one 4KB descriptor per partition); per gather, ACT
  computes 3 of the 4 corner products (ACTIVATE Copy with per-partition
  scale) while DVE does one fused multiply-accumulate plus 3 adds into a
  [128, 49*256] accumulator (the 0.25 avg-pool factor and the validity mask
  are folded into the weights); finished bins stored in chunks so the
  writeback overlaps the gather loop.
"""

import os
import sys

if "/opt/trn_rl_repo" not in sys.path:
    sys.path.insert(0, "/opt/trn_rl_repo")

import json

import numpy as np

_LEVELS = [(4, 256), (8, 128), (16, 64), (32, 32), (64, 16)]  # (stride, hw) for p2..p6
_BASES = [0, 65536, 81920, 86016, 87040]
_NPOS = 87296
_C = 256
_B, _R = 2, 512
_RPC = 128  # RoIs per core
_NCORES = 8
_NPTS = 196  # 7*7 bins * 2*2 sample points

_MAX_WAITS = 1


def _patch_json_bytes(raw: bytes) -> bytes:
    """walrus codegen on this stack accepts at most one sync wait per
    instruction; hoist excess waits onto preceding EventSemaphore carriers
    (same engine, program order => identical wait semantics)."""
    j = json.loads(raw)
    ctr = 0
    changed = False
    for f in j.get("functions", []):
        for blk in f.get("blocks", []):
            out = []
            for ins in blk.get("instructions", []):
                si = ins.get("sync_info")
                waits = si.get("on_wait") if si else None
                if waits and len(waits) > _MAX_WAITS:
                    changed = True
                    extra = waits[:-_MAX_WAITS]
                    si["on_wait"] = waits[-_MAX_WAITS:]
                    for i in range(0, len(extra), _MAX_WAITS):
                        ctr += 1
                        carrier = {
                            "engine": ins["engine"],
                            "ins": [],
                            "name": f"waitfix-{ctr}",
                            "opcode": "EventSemaphore",
                            "outs": [],
                            "sync_info": {
                                "on_update": [],
                                "on_wait": extra[i : i + _MAX_WAITS],
                            },
                        }
                        if "debug" in ins:
                            carrier["debug"] = ins["debug"]
                        out.append(carrier)
                out.append(ins)
            blk["instructions"] = out
    return json.dumps(j).encode() if changed else raw


_nc_cache = [None]


def _build_nc():
    if _nc_cache[0] is not None:
        return _nc_cache[0]
    import concourse.bass as bass
    import concourse.mybir as mybir
    import concourse.tile as tile

    AO = mybir.AluOpType
    f32 = mybir.dt.float32
    i32 = mybir.dt.int32

    nc = bass.Bass()
    featp = nc.dram_tensor("featp", [_NPOS, 2 * _C], f32, kind="ExternalInput")
    prop = nc.dram_tensor("prop", [_RPC, 4], f32, kind="ExternalInput")
    out = nc.dram_tensor("out", [_RPC, 49 * _C], f32, kind="ExternalOutput")

    with tile.TileContext(nc) as tc:
        with (
            tc.tile_pool(name="pre", bufs=1) as pre,
            tc.tile_pool(name="gp", bufs=10) as gp,
            tc.tile_pool(name="accp", bufs=1) as accp,
            tc.tile_pool(name="actp", bufs=6) as actp,
        ):
            P = _RPC
            pr = pre.tile([P, 4], f32)
            nc.sync.dma_start(out=pr[:], in_=prop[:])

            # cy4 = 4i + 2a + 1, cx4 = 4j + 2b + 1 over s=(((i*7+j)*2+a)*2+b)
            cy4i = pre.tile([P, _NPTS], i32)
            cx4i = pre.tile([P, _NPTS], i32)
            nc.gpsimd.iota(cy4i[:], pattern=[[4, 7], [0, 7], [2, 2], [0, 2]], base=1, channel_multiplier=0)
            nc.gpsimd.iota(cx4i[:], pattern=[[0, 7], [4, 7], [0, 2], [2, 2]], base=1, channel_multiplier=0)
            cy4 = pre.tile([P, _NPTS], f32)
            cx4 = pre.tile([P, _NPTS], f32)
            nc.vector.tensor_copy(out=cy4[:], in_=cy4i[:])
            nc.vector.tensor_copy(out=cx4[:], in_=cx4i[:])

            def col(name):
                return pre.tile([P, 1], f32, name=name, tag=name)

            x1, y1, x2, y2 = pr[:, 0:1], pr[:, 1:2], pr[:, 2:3], pr[:, 3:4]
            wim, him, area = col("wim"), col("him"), col("area")
            nc.vector.tensor_sub(wim[:], x2, x1)
            nc.vector.tensor_scalar_max(wim[:], wim[:], 1.0)
            nc.vector.tensor_sub(him[:], y2, y1)
            nc.vector.tensor_scalar_max(him[:], him[:], 1.0)
            nc.vector.tensor_mul(area[:], wim[:], him[:])

            # level masks by exact thresholds on area: lvl>=3 iff area>=112^2 etc.
            ths = [112.0**2, 224.0**2, 448.0**2, 896.0**2]
            tt_ = [col(f"t{k}") for k in range(4)]
            for k in range(4):
                nc.vector.tensor_scalar(out=tt_[k][:], in0=area[:], scalar1=ths[k], scalar2=None, op0=AO.is_ge)
            masks = [col(f"m{l}") for l in range(5)]  # m[0] = level2 ... m[4] = level6
            nc.vector.tensor_scalar(out=masks[0][:], in0=tt_[0][:], scalar1=-1.0, scalar2=1.0, op0=AO.mult, op1=AO.add)
            for k in range(3):
                nc.vector.tensor_sub(masks[k + 1][:], tt_[k][:], tt_[k + 1][:])
            masks[4] = tt_[3]

            def msel(name, consts):
                r = col(name)
                nc.vector.tensor_scalar(out=r[:], in0=masks[0][:], scalar1=consts[0], scalar2=None, op0=AO.mult)
                for k in range(1, 5):
                    nc.vector.scalar_tensor_tensor(
                        out=r[:], in0=masks[k][:], scalar=consts[k], in1=r[:], op0=AO.mult, op1=AO.add
                    )
                return r

            scale = msel("scale", [1.0 / s for s, _ in _LEVELS])
            wf = msel("wf", [float(hw) for _, hw in _LEVELS])
            basef = msel("basef", [float(b) for b in _BASES])
            wm1, wm2 = col("wm1"), col("wm2")
            nc.vector.tensor_scalar_add(wm1[:], wf[:], -1.0)
            nc.vector.tensor_scalar_add(wm2[:], wf[:], -2.0)

            pf = pre.tile([P, 4], f32)
            nc.vector.tensor_scalar_mul(pf[:], pr[:], scale[:])
            x1f, y1f, x2f, y2f = pf[:, 0:1], pf[:, 1:2], pf[:, 2:3], pf[:, 3:4]
            roiw, roih, bw4, bh4 = col("roiw"), col("roih"), col("bw4"), col("bh4")
            nc.vector.tensor_sub(roiw[:], x2f, x1f)
            nc.vector.tensor_scalar_max(roiw[:], roiw[:], 1.0)
            nc.vector.tensor_sub(roih[:], y2f, y1f)
            nc.vector.tensor_scalar_max(roih[:], roih[:], 1.0)
            nc.vector.tensor_scalar_mul(bw4[:], roiw[:], 1.0 / 28.0)
            nc.vector.tensor_scalar_mul(bh4[:], roih[:], 1.0 / 28.0)

            def pts(name):
                return pre.tile([P, _NPTS], f32, name=name, tag=name)

            ys, xs = pts("ys"), pts("xs")
            nc.vector.tensor_scalar(out=ys[:], in0=cy4[:], scalar1=bh4[:], scalar2=y1f, op0=AO.mult, op1=AO.add)
            nc.vector.tensor_scalar(out=xs[:], in0=cx4[:], scalar1=bw4[:], scalar2=x1f, op0=AO.mult, op1=AO.add)

            yc, xc = pts("yc"), pts("xc")
            nc.vector.tensor_scalar(out=yc[:], in0=ys[:], scalar1=0.0, scalar2=wm1[:], op0=AO.max, op1=AO.min)
            nc.vector.tensor_scalar(out=xc[:], in0=xs[:], scalar1=0.0, scalar2=wm1[:], op0=AO.max, op1=AO.min)

            def floor_clamp(src, name):
                # y0p = clamp(round(y - 0.5), 0, W-2); round-at-tie either way is
                # correct for bilinear interp thanks to the clamps.
                sh = pts(name + "sh")
                ii = pre.tile([P, _NPTS], i32, tag=name + "i")
                ff = pts(name + "f")
                nc.vector.tensor_scalar_add(sh[:], src[:], -0.5)
                nc.vector.tensor_copy(out=ii[:], in_=sh[:])
                nc.vector.tensor_copy(out=ff[:], in_=ii[:])
                nc.vector.tensor_scalar(out=ff[:], in0=ff[:], scalar1=0.0, scalar2=wm2[:], op0=AO.max, op1=AO.min)
                return ff

            y0p = floor_clamp(yc, "y0")
            x0p = floor_clamp(xc, "x0")

            # gather indices first, in program order, so the indirect DMA stream
            # can start while the weights below are still being computed
            posf = pts("posf")
            nc.vector.tensor_scalar(out=posf[:], in0=y0p[:], scalar1=wf[:], scalar2=basef[:], op0=AO.mult, op1=AO.add)
            nc.vector.tensor_add(posf[:], posf[:], x0p[:])
            idx = pre.tile([P, _NPTS], i32)
            nc.vector.tensor_copy(out=idx[:], in_=posf[:])

            valid, vtmp = pts("valid"), pts("vtmp")
            nc.vector.tensor_scalar(out=valid[:], in0=ys[:], scalar1=-1.0, scalar2=None, op0=AO.is_ge)
            nc.vector.tensor_scalar(out=vtmp[:], in0=ys[:], scalar1=wf[:], scalar2=None, op0=AO.is_le)
            nc.vector.tensor_mul(valid[:], valid[:], vtmp[:])
            nc.vector.tensor_scalar(out=vtmp[:], in0=xs[:], scalar1=-1.0, scalar2=None, op0=AO.is_ge)
            nc.vector.tensor_mul(valid[:], valid[:], vtmp[:])
            nc.vector.tensor_scalar(out=vtmp[:], in0=xs[:], scalar1=wf[:], scalar2=None, op0=AO.is_le)
            nc.vector.tensor_mul(valid[:], valid[:], vtmp[:])
            # fold avg-pool 1/4 into the validity factor
            nc.vector.tensor_scalar_mul(valid[:], valid[:], 0.25)

            ly, hy, lx, hx = pts("ly"), pts("hy"), pts("lx"), pts("hx")
            nc.vector.tensor_sub(ly[:], yc[:], y0p[:])
            nc.vector.tensor_scalar(out=hy[:], in0=ly[:], scalar1=-1.0, scalar2=1.0, op0=AO.mult, op1=AO.add)
            nc.vector.tensor_sub(lx[:], xc[:], x0p[:])
            nc.vector.tensor_scalar(out=hx[:], in0=lx[:], scalar1=-1.0, scalar2=1.0, op0=AO.mult, op1=AO.add)

            hyq, lyq = pts("hyq"), pts("lyq")
            nc.vector.tensor_mul(hyq[:], hy[:], valid[:])
            nc.vector.tensor_mul(lyq[:], ly[:], valid[:])
            w00, w10, w01, w11 = pts("w00"), pts("w10"), pts("w01"), pts("w11")
            nc.vector.tensor_mul(w00[:], hyq[:], hx[:])
            nc.vector.tensor_mul(w10[:], lyq[:], hx[:])
            nc.vector.tensor_mul(w01[:], hyq[:], lx[:])
            nc.vector.tensor_mul(w11[:], lyq[:], lx[:])

            acc = accp.tile([P, 49 * _C], f32)
            from concourse.bass import IndirectOffsetOnAxis

            for t in range(_NPTS):
                g = gp.tile([P, 4 * _C], f32, tag="g")
                nc.gpsimd.indirect_dma_start(
                    out=g[:],
                    out_offset=None,
                    in_=featp[:],
                    in_offset=IndirectOffsetOnAxis(ap=idx[:, t : t + 1], axis=0),
                )
                b = t // 4
                sl = acc[:, b * _C : (b + 1) * _C]
                # ACT (otherwise idle) computes the first corner product of every
                # tile; DVE then pays a cheaper tensor-add instead of one of its
                # 1x-rate scalar_tensor_tensor ops.
                p = actp.tile([P, _C], f32, tag="actp")
                nc.scalar.activation(
                    out=p[:], in_=g[:, 0:256], func=mybir.ActivationFunctionType.Copy,
                    scale=w00[:, t : t + 1],
                )
                p2 = actp.tile([P, _C], f32, tag="actp2")
                nc.scalar.activation(
                    out=p2[:], in_=g[:, 512:768], func=mybir.ActivationFunctionType.Copy,
                    scale=w01[:, t : t + 1],
                )
                if t % 4 == 0:
                    nc.vector.scalar_tensor_tensor(
                        out=sl, in0=g[:, 256:512], scalar=w10[:, t : t + 1], in1=p[:], op0=AO.mult, op1=AO.add
                    )
                else:
                    nc.vector.tensor_add(sl, sl, p[:])
                    nc.vector.scalar_tensor_tensor(
                        out=sl, in0=g[:, 256:512], scalar=w10[:, t : t + 1], in1=sl, op0=AO.mult, op1=AO.add
                    )
                p3 = actp.tile([P, _C], f32, tag="actp3")
                nc.scalar.activation(
                    out=p3[:], in_=g[:, 768:1024], func=mybir.ActivationFunctionType.Copy,
                    scale=w11[:, t : t + 1],
                )
                nc.vector.tensor_add(sl, sl, p2[:])
                nc.gpsimd.tensor_add(sl, sl, p3[:])
                # store finished bins in chunks of 7 so the writeback overlaps the loop
                if t % 28 == 27:
                    ch = t // 28
                    nc.sync.dma_start(
                        out=out[:, ch * 7 * _C : (ch + 1) * 7 * _C],
                        in_=acc[:, ch * 7 * _C : (ch + 1) * 7 * _C],
                    )

    nc.finalize()
    orig = nc.to_json_bytes
    nc.to_json_bytes = lambda: _patch_json_bytes(orig())
    _nc_cache[0] = nc
    return nc


def _build_featp(p2, p3, p4, p5, p6):
    """[B, NPOS, 512] row-pair concat pyramid."""
    parts = []
    for arr in (p2, p3, p4, p5, p6):
        B, H, W, C = arr.shape
        nxt = arr[:, np.minimum(np.arange(H) + 1, H - 1), :, :]
        pair = np.concatenate([arr, nxt], axis=-1)  # [B, H, W, 2C]
        parts.append(pair.reshape(B, H * W, 2 * C))
    return np.ascontiguousarray(np.concatenate(parts, axis=1), dtype=np.float32)


def kernel(p2, p3, p4, p5, p6, proposals):
    from concourse.bass_utils import run_bass_kernel_spmd

    nc = _build_nc()
    featp = _build_featp(
        np.asarray(p2, dtype=np.float32),
        np.asarray(p3, dtype=np.float32),
        np.asarray(p4, dtype=np.float32),
        np.asarray(p5, dtype=np.float32),
        np.asarray(p6, dtype=np.float32),
    )
    props = np.ascontiguousarray(np.asarray(proposals, dtype=np.float32))

    in_maps = []
    for c in range(_NCORES):
        b, blk = divmod(c, _NCORES // _B)
        in_maps.append(
            {
                "featp": featp[b],
                "prop": np.ascontiguousarray(props[b, blk * _RPC : (blk + 1) * _RPC]),
            }
        )

    trace = bool(os.environ.get("BASS_ROI_TRACE"))
    kwargs = {}
    if trace:
        kwargs = {"trace": True, "tmpdir": os.environ.get("BASS_ROI_TRACE_DIR") or None}
    res = run_bass_kernel_spmd(nc, in_maps, core_ids=list(range(_NCORES)), **kwargs)
    if trace:
        print(f"HW exec time: {res.exec_time_ns} ns")
        if res.instructions_and_trace:
            print("trace:", res.instructions_and_trace[1])

    out = np.empty((_B, _R, 7, 7, _C), dtype=np.float32)
    for c in range(_NCORES):
        b, blk = divmod(c, _NCORES // _B)
        out[b, blk * _RPC : (blk + 1) * _RPC] = res.results[c]["out"].reshape(_RPC, 7, 7, _C)
    return out


# revision 22
# speedup vs baseline: 1.4084x; 1.4084x over previous
"""Multi-level RoIAlign (FPN BaseRoIHead) as a Trainium2 Bass kernel.

Contract: kernel(**inputs) takes the FULL unsharded inputs
(p2..p6: [2,H,W,256] f32, proposals: [2,512,4] f32) and returns the FULL
output [2, 512, 7, 7, 256] f32.

Strategy:
- Shard the 2*512 RoIs over 8 cores (128 RoIs each; cores 0-3 image 0,
  cores 4-7 image 1). Each core receives its image's feature pyramid in a
  "row-pair" layout: featp[base_l + y*W + x] = concat(feat[y,x,:], feat[y+1,x,:])
  so that the 4 bilinear corners of one sample point are ONE contiguous 4KB
  block (positions (y0,x0) and (y0,x0+1)).
- On device: per-RoI level assignment via exact area thresholds, sample
  coordinates / bilinear weights / gather indices computed on DVE in a
  [128 RoIs x 196 sample points] layout; 196 indirect DMA gathers of
  [128, 1024] f32 (one 4KB descriptor per partition); per gather, ACT
  computes 3 of the 4 corner products (ACTIVATE Copy with per-partition
  scale) while DVE does one fused multiply-accumulate plus 3 adds into a
  [128, 49*256] accumulator (the 0.25 avg-pool factor and the validity mask
  are folded into the weights); finished bins stored in chunks so the
  writeback overlaps the gather loop.
"""

import os
import sys

if "/opt/trn_rl_repo" not in sys.path:
    sys.path.insert(0, "/opt/trn_rl_repo")

import json

import numpy as np

_LEVELS = [(4, 256), (8, 128), (16, 64), (32, 32), (64, 16)]  # (stride, hw) for p2..p6
_BASES = [0, 65536, 81920, 86016, 87040]
_NPOS = 87296
_C = 256
_B, _R = 2, 512
_RPC = 128  # RoIs per core
_NCORES = 8
_NPTS = 196  # 7*7 bins * 2*2 sample points

_MAX_WAITS = 1


def _patch_json_bytes(raw: bytes) -> bytes:
    """walrus codegen on this stack accepts at most one sync wait per
    instruction; hoist excess waits onto preceding EventSemaphore carriers
    (same engine, program order => identical wait semantics)."""
    j = json.loads(raw)
    ctr = 0
    changed = False
    for f in j.get("functions", []):
        for blk in f.get("blocks", []):
            out = []
            for ins in blk.get("instructions", []):
                si = ins.get("sync_info")
                waits = si.get("on_wait") if si else None
                if waits and len(waits) > _MAX_WAITS:
                    changed = True
                    extra = waits[:-_MAX_WAITS]
                    si["on_wait"] = waits[-_MAX_WAITS:]
                    for i in range(0, len(extra), _MAX_WAITS):
                        ctr += 1
                        carrier = {
                            "engine": ins["engine"],
                            "ins": [],
                            "name": f"waitfix-{ctr}",
                            "opcode": "EventSemaphore",
                            "outs": [],
                            "sync_info": {
                                "on_update": [],
                                "on_wait": extra[i : i + _MAX_WAITS],
                            },
                        }
                        if "debug" in ins:
                            carrier["debug"] = ins["debug"]
                        out.append(carrier)
                out.append(ins)
            blk["instructions"] = out
    return json.dumps(j).encode() if changed else raw


_nc_cache = [None]


def _build_nc():
    if _nc_cache[0] is not None:
        return _nc_cache[0]
    import concourse.bass as bass
    import concourse.mybir as mybir
    import concourse.tile as tile

    AO = mybir.AluOpType
    f32 = mybir.dt.float32
    i32 = mybir.dt.int32

    nc = bass.Bass()
    featp = nc.dram_tensor("featp", [_NPOS, 2 * _C], f32, kind="ExternalInput")
    prop = nc.dram_tensor("prop", [_RPC, 4], f32, kind="ExternalInput")
    out = nc.dram_tensor("out", [_RPC, 49 * _C], f32, kind="ExternalOutput")

    with tile.TileContext(nc) as tc:
        with (
            tc.tile_pool(name="pre", bufs=1) as pre,
            tc.tile_pool(name="gp", bufs=10) as gp,
            tc.tile_pool(name="accp", bufs=1) as accp,
            tc.tile_pool(name="actp", bufs=6) as actp,
        ):
            P = _RPC
            pr = pre.tile([P, 4], f32)
            nc.sync.dma_start(out=pr[:], in_=prop[:])

            # cy4 = 4i + 2a + 1, cx4 = 4j + 2b + 1 over s=(((i*7+j)*2+a)*2+b)
            cy4i = pre.tile([P, _NPTS], i32)
            cx4i = pre.tile([P, _NPTS], i32)
            nc.gpsimd.iota(cy4i[:], pattern=[[4, 7], [0, 7], [2, 2], [0, 2]], base=1, channel_multiplier=0)
            nc.gpsimd.iota(cx4i[:], pattern=[[0, 7], [4, 7], [0, 2], [2, 2]], base=1, channel_multiplier=0)
            cy4 = pre.tile([P, _NPTS], f32)
            cx4 = pre.tile([P, _NPTS], f32)
            nc.vector.tensor_copy(out=cy4[:], in_=cy4i[:])
            nc.vector.tensor_copy(out=cx4[:], in_=cx4i[:])

            def col(name):
                return pre.tile([P, 1], f32, name=name, tag=name)

            x1, y1, x2, y2 = pr[:, 0:1], pr[:, 1:2], pr[:, 2:3], pr[:, 3:4]
            wim, him, area = col("wim"), col("him"), col("area")
            nc.vector.tensor_sub(wim[:], x2, x1)
            nc.vector.tensor_scalar_max(wim[:], wim[:], 1.0)
            nc.vector.tensor_sub(him[:], y2, y1)
            nc.vector.tensor_scalar_max(him[:], him[:], 1.0)
            nc.vector.tensor_mul(area[:], wim[:], him[:])

            # level masks by exact thresholds on area: lvl>=3 iff area>=112^2 etc.
            ths = [112.0**2, 224.0**2, 448.0**2, 896.0**2]
            tt_ = [col(f"t{k}") for k in range(4)]
            for k in range(4):
                nc.vector.tensor_scalar(out=tt_[k][:], in0=area[:], scalar1=ths[k], scalar2=None, op0=AO.is_ge)
            masks = [col(f"m{l}") for l in range(5)]  # m[0] = level2 ... m[4] = level6
            nc.vector.tensor_scalar(out=masks[0][:], in0=tt_[0][:], scalar1=-1.0, scalar2=1.0, op0=AO.mult, op1=AO.add)
            for k in range(3):
                nc.vector.tensor_sub(masks[k + 1][:], tt_[k][:], tt_[k + 1][:])
            masks[4] = tt_[3]

            def msel(name, consts):
                r = col(name)
                nc.vector.tensor_scalar(out=r[:], in0=masks[0][:], scalar1=consts[0], scalar2=None, op0=AO.mult)
                for k in range(1, 5):
                    nc.vector.scalar_tensor_tensor(
                        out=r[:], in0=masks[k][:], scalar=consts[k], in1=r[:], op0=AO.mult, op1=AO.add
                    )
                return r

            scale = msel("scale", [1.0 / s for s, _ in _LEVELS])
            wf = msel("wf", [float(hw) for _, hw in _LEVELS])
            basef = msel("basef", [float(b) for b in _BASES])
            wm1, wm2 = col("wm1"), col("wm2")
            nc.vector.tensor_scalar_add(wm1[:], wf[:], -1.0)
            nc.vector.tensor_scalar_add(wm2[:], wf[:], -2.0)

            pf = pre.tile([P, 4], f32)
            nc.vector.tensor_scalar_mul(pf[:], pr[:], scale[:])
            x1f, y1f, x2f, y2f = pf[:, 0:1], pf[:, 1:2], pf[:, 2:3], pf[:, 3:4]
            roiw, roih, bw4, bh4 = col("roiw"), col("roih"), col("bw4"), col("bh4")
            nc.vector.tensor_sub(roiw[:], x2f, x1f)
            nc.vector.tensor_scalar_max(roiw[:], roiw[:], 1.0)
            nc.vector.tensor_sub(roih[:], y2f, y1f)
            nc.vector.tensor_scalar_max(roih[:], roih[:], 1.0)
            nc.vector.tensor_scalar_mul(bw4[:], roiw[:], 1.0 / 28.0)
            nc.vector.tensor_scalar_mul(bh4[:], roih[:], 1.0 / 28.0)

            def pts(name):
                return pre.tile([P, _NPTS], f32, name=name, tag=name)

            ys, xs = pts("ys"), pts("xs")
            nc.vector.tensor_scalar(out=ys[:], in0=cy4[:], scalar1=bh4[:], scalar2=y1f, op0=AO.mult, op1=AO.add)
            nc.vector.tensor_scalar(out=xs[:], in0=cx4[:], scalar1=bw4[:], scalar2=x1f, op0=AO.mult, op1=AO.add)

            yc, xc = pts("yc"), pts("xc")
            nc.vector.tensor_scalar(out=yc[:], in0=ys[:], scalar1=0.0, scalar2=wm1[:], op0=AO.max, op1=AO.min)
            nc.vector.tensor_scalar(out=xc[:], in0=xs[:], scalar1=0.0, scalar2=wm1[:], op0=AO.max, op1=AO.min)

            def floor_clamp(src, name):
                # y0p = clamp(round(y - 0.5), 0, W-2); round-at-tie either way is
                # correct for bilinear interp thanks to the clamps.
                sh = pts(name + "sh")
                ii = pre.tile([P, _NPTS], i32, tag=name + "i")
                ff = pts(name + "f")
                nc.vector.tensor_scalar_add(sh[:], src[:], -0.5)
                nc.vector.tensor_copy(out=ii[:], in_=sh[:])
                nc.vector.tensor_copy(out=ff[:], in_=ii[:])
                nc.vector.tensor_scalar(out=ff[:], in0=ff[:], scalar1=0.0, scalar2=wm2[:], op0=AO.max, op1=AO.min)
                return ff

            y0p = floor_clamp(yc, "y0")
            x0p = floor_clamp(xc, "x0")

            # gather indices first, in program order, so the indirect DMA stream
            # can start while the weights below are still being computed
            posf = pts("posf")
            nc.vector.tensor_scalar(out=posf[:], in0=y0p[:], scalar1=wf[:], scalar2=basef[:], op0=AO.mult, op1=AO.add)
            nc.vector.tensor_add(posf[:], posf[:], x0p[:])
            idx = pre.tile([P, _NPTS], i32)
            nc.vector.tensor_copy(out=idx[:], in_=posf[:])

            valid, vtmp = pts("valid"), pts("vtmp")
            nc.vector.tensor_scalar(out=valid[:], in0=ys[:], scalar1=-1.0, scalar2=None, op0=AO.is_ge)
            nc.vector.tensor_scalar(out=vtmp[:], in0=ys[:], scalar1=wf[:], scalar2=None, op0=AO.is_le)
            nc.vector.tensor_mul(valid[:], valid[:], vtmp[:])
            nc.vector.tensor_scalar(out=vtmp[:], in0=xs[:], scalar1=-1.0, scalar2=None, op0=AO.is_ge)
            nc.vector.tensor_mul(valid[:], valid[:], vtmp[:])
            nc.vector.tensor_scalar(out=vtmp[:], in0=xs[:], scalar1=wf[:], scalar2=None, op0=AO.is_le)
            nc.vector.tensor_mul(valid[:], valid[:], vtmp[:])
            # fold avg-pool 1/4 into the validity factor
            nc.vector.tensor_scalar_mul(valid[:], valid[:], 0.25)

            ly, hy, lx, hx = pts("ly"), pts("hy"), pts("lx"), pts("hx")
            nc.vector.tensor_sub(ly[:], yc[:], y0p[:])
            nc.vector.tensor_scalar(out=hy[:], in0=ly[:], scalar1=-1.0, scalar2=1.0, op0=AO.mult, op1=AO.add)
            nc.vector.tensor_sub(lx[:], xc[:], x0p[:])
            nc.vector.tensor_scalar(out=hx[:], in0=lx[:], scalar1=-1.0, scalar2=1.0, op0=AO.mult, op1=AO.add)

            hyq, lyq = pts("hyq"), pts("lyq")
            nc.vector.tensor_mul(hyq[:], hy[:], valid[:])
            nc.vector.tensor_mul(lyq[:], ly[:], valid[:])
            w00, w10, w01, w11 = pts("w00"), pts("w10"), pts("w01"), pts("w11")
            nc.vector.tensor_mul(w00[:], hyq[:], hx[:])
            nc.vector.tensor_mul(w10[:], lyq[:], hx[:])
            nc.vector.tensor_mul(w01[:], hyq[:], lx[:])
            nc.vector.tensor_mul(w11[:], lyq[:], lx[:])

            acc = accp.tile([P, 49 * _C], f32)
            from concourse.bass import IndirectOffsetOnAxis

            for t in range(_NPTS):
                g = gp.tile([P, 4 * _C], f32, tag="g")
                nc.gpsimd.indirect_dma_start(
                    out=g[:],
                    out_offset=None,
                    in_=featp[:],
                    in_offset=IndirectOffsetOnAxis(ap=idx[:, t : t + 1], axis=0),
                )
                b = t // 4
                sl = acc[:, b * _C : (b + 1) * _C]
                # ACT (otherwise idle) computes the first corner product of every
                # tile; DVE then pays a cheaper tensor-add instead of one of its
                # 1x-rate scalar_tensor_tensor ops.
                p = actp.tile([P, _C], f32, tag="actp")
                nc.scalar.activation(
                    out=p[:], in_=g[:, 0:256], func=mybir.ActivationFunctionType.Copy,
                    scale=w00[:, t : t + 1],
                )
                p2 = actp.tile([P, _C], f32, tag="actp2")
                nc.scalar.activation(
                    out=p2[:], in_=g[:, 512:768], func=mybir.ActivationFunctionType.Copy,
                    scale=w01[:, t : t + 1],
                )
                if t % 4 == 0:
                    nc.vector.scalar_tensor_tensor(
                        out=sl, in0=g[:, 256:512], scalar=w10[:, t : t + 1], in1=p[:], op0=AO.mult, op1=AO.add
                    )
                else:
                    nc.vector.tensor_add(sl, sl, p[:])
                    nc.vector.scalar_tensor_tensor(
                        out=sl, in0=g[:, 256:512], scalar=w10[:, t : t + 1], in1=sl, op0=AO.mult, op1=AO.add
                    )
                p3 = actp.tile([P, _C], f32, tag="actp3")
                nc.scalar.activation(
                    out=p3[:], in_=g[:, 768:1024], func=mybir.ActivationFunctionType.Copy,
                    scale=w11[:, t : t + 1],
                )
                nc.vector.tensor_add(sl, sl, p2[:])
                nc.vector.tensor_add(sl, sl, p3[:])
                # store finished bins in chunks of 7 so the writeback overlaps the loop
                if t % 28 == 27:
                    ch = t // 28
                    nc.sync.dma_start(
                        out=out[:, ch * 7 * _C : (ch + 1) * 7 * _C],
                        in_=acc[:, ch * 7 * _C : (ch + 1) * 7 * _C],
                    )

    nc.finalize()
    orig = nc.to_json_bytes
    nc.to_json_bytes = lambda: _patch_json_bytes(orig())
    _nc_cache[0] = nc
    return nc


def _build_featp(p2, p3, p4, p5, p6):
    """[B, NPOS, 512] row-pair concat pyramid."""
    parts = []
    for arr in (p2, p3, p4, p5, p6):
        B, H, W, C = arr.shape
        nxt = arr[:, np.minimum(np.arange(H) + 1, H - 1), :, :]
        pair = np.concatenate([arr, nxt], axis=-1)  # [B, H, W, 2C]
        parts.append(pair.reshape(B, H * W, 2 * C))
    return np.ascontiguousarray(np.concatenate(parts, axis=1), dtype=np.float32)


def kernel(p2, p3, p4, p5, p6, proposals):
    from concourse.bass_utils import run_bass_kernel_spmd

    nc = _build_nc()
    featp = _build_featp(
        np.asarray(p2, dtype=np.float32),
        np.asarray(p3, dtype=np.float32),
        np.asarray(p4, dtype=np.float32),
        np.asarray(p5, dtype=np.float32),
        np.asarray(p6, dtype=np.float32),
    )
    props = np.ascontiguousarray(np.asarray(proposals, dtype=np.float32))

    in_maps = []
    for c in range(_NCORES):
        b, blk = divmod(c, _NCORES // _B)
        in_maps.append(
            {
                "featp": featp[b],
                "prop": np.ascontiguousarray(props[b, blk * _RPC : (blk + 1) * _RPC]),
            }
        )

    trace = bool(os.environ.get("BASS_ROI_TRACE"))
    kwargs = {}
    if trace:
        kwargs = {"trace": True, "tmpdir": os.environ.get("BASS_ROI_TRACE_DIR") or None}
    res = run_bass_kernel_spmd(nc, in_maps, core_ids=list(range(_NCORES)), **kwargs)
    if trace:
        print(f"HW exec time: {res.exec_time_ns} ns")
        if res.instructions_and_trace:
            print("trace:", res.instructions_and_trace[1])

    out = np.empty((_B, _R, 7, 7, _C), dtype=np.float32)
    for c in range(_NCORES):
        b, blk = divmod(c, _NCORES // _B)
        out[b, blk * _RPC : (blk + 1) * _RPC] = res.results[c]["out"].reshape(_RPC, 7, 7, _C)
    return out


# revision 23
# speedup vs baseline: 1.4172x; 1.0062x over previous
"""Multi-level RoIAlign (FPN BaseRoIHead) as a Trainium2 Bass kernel.

Contract: kernel(**inputs) takes the FULL unsharded inputs
(p2..p6: [2,H,W,256] f32, proposals: [2,512,4] f32) and returns the FULL
output [2, 512, 7, 7, 256] f32.

Strategy:
- Shard the 2*512 RoIs over 8 cores (128 RoIs each; cores 0-3 image 0,
  cores 4-7 image 1). Each core receives its image's feature pyramid in a
  "row-pair" layout: featp[base_l + y*W + x] = concat(feat[y,x,:], feat[y+1,x,:])
  so that the 4 bilinear corners of one sample point are ONE contiguous 4KB
  block (positions (y0,x0) and (y0,x0+1)).
- On device: per-RoI level assignment via exact area thresholds, sample
  coordinates / bilinear weights / gather indices computed on DVE in a
  [128 RoIs x 196 sample points] layout; 196 indirect DMA gathers of
  [128, 1024] f32 (one 4KB descriptor per partition); per gather, ACT
  computes 3 of the 4 corner products (ACTIVATE Copy with per-partition
  scale) while DVE does one fused multiply-accumulate plus 3 adds into a
  [128, 49*256] accumulator (the 0.25 avg-pool factor and the validity mask
  are folded into the weights); finished bins stored in chunks so the
  writeback overlaps the gather loop.
"""

import os
import sys

if "/opt/trn_rl_repo" not in sys.path:
    sys.path.insert(0, "/opt/trn_rl_repo")

import json

import numpy as np

_LEVELS = [(4, 256), (8, 128), (16, 64), (32, 32), (64, 16)]  # (stride, hw) for p2..p6
_BASES = [0, 65536, 81920, 86016, 87040]
_NPOS = 87296
_C = 256
_B, _R = 2, 512
_RPC = 128  # RoIs per core
_NCORES = 8
_NPTS = 196  # 7*7 bins * 2*2 sample points

_MAX_WAITS = 1


def _patch_json_bytes(raw: bytes) -> bytes:
    """walrus codegen on this stack accepts at most one sync wait per
    instruction; hoist excess waits onto preceding EventSemaphore carriers
    (same engine, program order => identical wait semantics)."""
    j = json.loads(raw)
    ctr = 0
    changed = False
    for f in j.get("functions", []):
        for blk in f.get("blocks", []):
            out = []
            for ins in blk.get("instructions", []):
                si = ins.get("sync_info")
                waits = si.get("on_wait") if si else None
                if waits and len(waits) > _MAX_WAITS:
                    changed = True
                    extra = waits[:-_MAX_WAITS]
                    si["on_wait"] = waits[-_MAX_WAITS:]
                    for i in range(0, len(extra), _MAX_WAITS):
                        ctr += 1
                        carrier = {
                            "engine": ins["engine"],
                            "ins": [],
                            "name": f"waitfix-{ctr}",
                            "opcode": "EventSemaphore",
                            "outs": [],
                            "sync_info": {
                                "on_update": [],
                                "on_wait": extra[i : i + _MAX_WAITS],
                            },
                        }
                        if "debug" in ins:
                            carrier["debug"] = ins["debug"]
                        out.append(carrier)
                out.append(ins)
            blk["instructions"] = out
    return json.dumps(j).encode() if changed else raw


_nc_cache = [None]


def _build_nc():
    if _nc_cache[0] is not None:
        return _nc_cache[0]
    import concourse.bass as bass
    import concourse.mybir as mybir
    import concourse.tile as tile

    AO = mybir.AluOpType
    f32 = mybir.dt.float32
    i32 = mybir.dt.int32

    nc = bass.Bass()
    featp = nc.dram_tensor("featp", [_NPOS, 2 * _C], f32, kind="ExternalInput")
    prop = nc.dram_tensor("prop", [_RPC, 4], f32, kind="ExternalInput")
    out = nc.dram_tensor("out", [_RPC, 49 * _C], f32, kind="ExternalOutput")

    with tile.TileContext(nc) as tc:
        with (
            tc.tile_pool(name="pre", bufs=1) as pre,
            tc.tile_pool(name="gp", bufs=10) as gp,
            tc.tile_pool(name="accp", bufs=1) as accp,
            tc.tile_pool(name="actp", bufs=12) as actp,
        ):
            P = _RPC
            pr = pre.tile([P, 4], f32)
            nc.sync.dma_start(out=pr[:], in_=prop[:])

            # cy4 = 4i + 2a + 1, cx4 = 4j + 2b + 1 over s=(((i*7+j)*2+a)*2+b)
            cy4i = pre.tile([P, _NPTS], i32)
            cx4i = pre.tile([P, _NPTS], i32)
            nc.gpsimd.iota(cy4i[:], pattern=[[4, 7], [0, 7], [2, 2], [0, 2]], base=1, channel_multiplier=0)
            nc.gpsimd.iota(cx4i[:], pattern=[[0, 7], [4, 7], [0, 2], [2, 2]], base=1, channel_multiplier=0)
            cy4 = pre.tile([P, _NPTS], f32)
            cx4 = pre.tile([P, _NPTS], f32)
            nc.vector.tensor_copy(out=cy4[:], in_=cy4i[:])
            nc.vector.tensor_copy(out=cx4[:], in_=cx4i[:])

            def col(name):
                return pre.tile([P, 1], f32, name=name, tag=name)

            x1, y1, x2, y2 = pr[:, 0:1], pr[:, 1:2], pr[:, 2:3], pr[:, 3:4]
            wim, him, area = col("wim"), col("him"), col("area")
            nc.vector.tensor_sub(wim[:], x2, x1)
            nc.vector.tensor_scalar_max(wim[:], wim[:], 1.0)
            nc.vector.tensor_sub(him[:], y2, y1)
            nc.vector.tensor_scalar_max(him[:], him[:], 1.0)
            nc.vector.tensor_mul(area[:], wim[:], him[:])

            # level masks by exact thresholds on area: lvl>=3 iff area>=112^2 etc.
            ths = [112.0**2, 224.0**2, 448.0**2, 896.0**2]
            tt_ = [col(f"t{k}") for k in range(4)]
            for k in range(4):
                nc.vector.tensor_scalar(out=tt_[k][:], in0=area[:], scalar1=ths[k], scalar2=None, op0=AO.is_ge)
            masks = [col(f"m{l}") for l in range(5)]  # m[0] = level2 ... m[4] = level6
            nc.vector.tensor_scalar(out=masks[0][:], in0=tt_[0][:], scalar1=-1.0, scalar2=1.0, op0=AO.mult, op1=AO.add)
            for k in range(3):
                nc.vector.tensor_sub(masks[k + 1][:], tt_[k][:], tt_[k + 1][:])
            masks[4] = tt_[3]

            def msel(name, consts):
                r = col(name)
                nc.vector.tensor_scalar(out=r[:], in0=masks[0][:], scalar1=consts[0], scalar2=None, op0=AO.mult)
                for k in range(1, 5):
                    nc.vector.scalar_tensor_tensor(
                        out=r[:], in0=masks[k][:], scalar=consts[k], in1=r[:], op0=AO.mult, op1=AO.add
                    )
                return r

            scale = msel("scale", [1.0 / s for s, _ in _LEVELS])
            wf = msel("wf", [float(hw) for _, hw in _LEVELS])
            basef = msel("basef", [float(b) for b in _BASES])
            wm1, wm2 = col("wm1"), col("wm2")
            nc.vector.tensor_scalar_add(wm1[:], wf[:], -1.0)
            nc.vector.tensor_scalar_add(wm2[:], wf[:], -2.0)

            pf = pre.tile([P, 4], f32)
            nc.vector.tensor_scalar_mul(pf[:], pr[:], scale[:])
            x1f, y1f, x2f, y2f = pf[:, 0:1], pf[:, 1:2], pf[:, 2:3], pf[:, 3:4]
            roiw, roih, bw4, bh4 = col("roiw"), col("roih"), col("bw4"), col("bh4")
            nc.vector.tensor_sub(roiw[:], x2f, x1f)
            nc.vector.tensor_scalar_max(roiw[:], roiw[:], 1.0)
            nc.vector.tensor_sub(roih[:], y2f, y1f)
            nc.vector.tensor_scalar_max(roih[:], roih[:], 1.0)
            nc.vector.tensor_scalar_mul(bw4[:], roiw[:], 1.0 / 28.0)
            nc.vector.tensor_scalar_mul(bh4[:], roih[:], 1.0 / 28.0)

            def pts(name):
                return pre.tile([P, _NPTS], f32, name=name, tag=name)

            ys, xs = pts("ys"), pts("xs")
            nc.vector.tensor_scalar(out=ys[:], in0=cy4[:], scalar1=bh4[:], scalar2=y1f, op0=AO.mult, op1=AO.add)
            nc.vector.tensor_scalar(out=xs[:], in0=cx4[:], scalar1=bw4[:], scalar2=x1f, op0=AO.mult, op1=AO.add)

            yc, xc = pts("yc"), pts("xc")
            nc.vector.tensor_scalar(out=yc[:], in0=ys[:], scalar1=0.0, scalar2=wm1[:], op0=AO.max, op1=AO.min)
            nc.vector.tensor_scalar(out=xc[:], in0=xs[:], scalar1=0.0, scalar2=wm1[:], op0=AO.max, op1=AO.min)

            def floor_clamp(src, name):
                # y0p = clamp(round(y - 0.5), 0, W-2); round-at-tie either way is
                # correct for bilinear interp thanks to the clamps.
                sh = pts(name + "sh")
                ii = pre.tile([P, _NPTS], i32, tag=name + "i")
                ff = pts(name + "f")
                nc.vector.tensor_scalar_add(sh[:], src[:], -0.5)
                nc.vector.tensor_copy(out=ii[:], in_=sh[:])
                nc.vector.tensor_copy(out=ff[:], in_=ii[:])
                nc.vector.tensor_scalar(out=ff[:], in0=ff[:], scalar1=0.0, scalar2=wm2[:], op0=AO.max, op1=AO.min)
                return ff

            y0p = floor_clamp(yc, "y0")
            x0p = floor_clamp(xc, "x0")

            # gather indices first, in program order, so the indirect DMA stream
            # can start while the weights below are still being computed
            posf = pts("posf")
            nc.vector.tensor_scalar(out=posf[:], in0=y0p[:], scalar1=wf[:], scalar2=basef[:], op0=AO.mult, op1=AO.add)
            nc.vector.tensor_add(posf[:], posf[:], x0p[:])
            idx = pre.tile([P, _NPTS], i32)
            nc.vector.tensor_copy(out=idx[:], in_=posf[:])

            valid, vtmp = pts("valid"), pts("vtmp")
            nc.vector.tensor_scalar(out=valid[:], in0=ys[:], scalar1=-1.0, scalar2=None, op0=AO.is_ge)
            nc.vector.tensor_scalar(out=vtmp[:], in0=ys[:], scalar1=wf[:], scalar2=None, op0=AO.is_le)
            nc.vector.tensor_mul(valid[:], valid[:], vtmp[:])
            nc.vector.tensor_scalar(out=vtmp[:], in0=xs[:], scalar1=-1.0, scalar2=None, op0=AO.is_ge)
            nc.vector.tensor_mul(valid[:], valid[:], vtmp[:])
            nc.vector.tensor_scalar(out=vtmp[:], in0=xs[:], scalar1=wf[:], scalar2=None, op0=AO.is_le)
            nc.vector.tensor_mul(valid[:], valid[:], vtmp[:])
            # fold avg-pool 1/4 into the validity factor
            nc.vector.tensor_scalar_mul(valid[:], valid[:], 0.25)

            ly, hy, lx, hx = pts("ly"), pts("hy"), pts("lx"), pts("hx")
            nc.vector.tensor_sub(ly[:], yc[:], y0p[:])
            nc.vector.tensor_scalar(out=hy[:], in0=ly[:], scalar1=-1.0, scalar2=1.0, op0=AO.mult, op1=AO.add)
            nc.vector.tensor_sub(lx[:], xc[:], x0p[:])
            nc.vector.tensor_scalar(out=hx[:], in0=lx[:], scalar1=-1.0, scalar2=1.0, op0=AO.mult, op1=AO.add)

            hyq, lyq = pts("hyq"), pts("lyq")
            nc.vector.tensor_mul(hyq[:], hy[:], valid[:])
            nc.vector.tensor_mul(lyq[:], ly[:], valid[:])
            w00, w10, w01, w11 = pts("w00"), pts("w10"), pts("w01"), pts("w11")
            nc.vector.tensor_mul(w00[:], hyq[:], hx[:])
            nc.vector.tensor_mul(w10[:], lyq[:], hx[:])
            nc.vector.tensor_mul(w01[:], hyq[:], lx[:])
            nc.vector.tensor_mul(w11[:], lyq[:], lx[:])

            acc = accp.tile([P, 49 * _C], f32)
            from concourse.bass import IndirectOffsetOnAxis

            for t in range(_NPTS):
                g = gp.tile([P, 4 * _C], f32, tag="g")
                nc.gpsimd.indirect_dma_start(
                    out=g[:],
                    out_offset=None,
                    in_=featp[:],
                    in_offset=IndirectOffsetOnAxis(ap=idx[:, t : t + 1], axis=0),
                )
                b = t // 4
                sl = acc[:, b * _C : (b + 1) * _C]
                # ACT (otherwise idle) computes the first corner product of every
                # tile; DVE then pays a cheaper tensor-add instead of one of its
                # 1x-rate scalar_tensor_tensor ops.
                p = actp.tile([P, _C], f32, tag="actp")
                nc.scalar.activation(
                    out=p[:], in_=g[:, 0:256], func=mybir.ActivationFunctionType.Copy,
                    scale=w00[:, t : t + 1],
                )
                p2 = actp.tile([P, _C], f32, tag="actp2")
                nc.scalar.activation(
                    out=p2[:], in_=g[:, 512:768], func=mybir.ActivationFunctionType.Copy,
                    scale=w01[:, t : t + 1],
                )
                if t % 4 == 0:
                    nc.vector.scalar_tensor_tensor(
                        out=sl, in0=g[:, 256:512], scalar=w10[:, t : t + 1], in1=p[:], op0=AO.mult, op1=AO.add
                    )
                else:
                    nc.vector.tensor_add(sl, sl, p[:])
                    nc.vector.scalar_tensor_tensor(
                        out=sl, in0=g[:, 256:512], scalar=w10[:, t : t + 1], in1=sl, op0=AO.mult, op1=AO.add
                    )
                p3 = actp.tile([P, _C], f32, tag="actp3")
                nc.scalar.activation(
                    out=p3[:], in_=g[:, 768:1024], func=mybir.ActivationFunctionType.Copy,
                    scale=w11[:, t : t + 1],
                )
                nc.vector.tensor_add(sl, sl, p2[:])
                nc.vector.tensor_add(sl, sl, p3[:])
                # store finished bins in chunks of 7 so the writeback overlaps the loop
                if t % 28 == 27:
                    ch = t // 28
                    nc.sync.dma_start(
                        out=out[:, ch * 7 * _C : (ch + 1) * 7 * _C],
                        in_=acc[:, ch * 7 * _C : (ch + 1) * 7 * _C],
                    )

    nc.finalize()
    orig = nc.to_json_bytes
    nc.to_json_bytes = lambda: _patch_json_bytes(orig())
    _nc_cache[0] = nc
    return nc


def _build_featp(p2, p3, p4, p5, p6):
    """[B, NPOS, 512] row-pair concat pyramid."""
    parts = []
    for arr in (p2, p3, p4, p5, p6):
        B, H, W, C = arr.shape
        nxt = arr[:, np.minimum(np.arange(H) + 1, H - 1), :, :]
        pair = np.concatenate([arr, nxt], axis=-1)  # [B, H, W, 2C]
        parts.append(pair.reshape(B, H * W, 2 * C))
    return np.ascontiguousarray(np.concatenate(parts, axis=1), dtype=np.float32)


def kernel(p2, p3, p4, p5, p6, proposals):
    from concourse.bass_utils import run_bass_kernel_spmd

    nc = _build_nc()
    featp = _build_featp(
        np.asarray(p2, dtype=np.float32),
        np.asarray(p3, dtype=np.float32),
        np.asarray(p4, dtype=np.float32),
        np.asarray(p5, dtype=np.float32),
        np.asarray(p6, dtype=np.float32),
    )
    props = np.ascontiguousarray(np.asarray(proposals, dtype=np.float32))

    in_maps = []
    for c in range(_NCORES):
        b, blk = divmod(c, _NCORES // _B)
        in_maps.append(
            {
                "featp": featp[b],
                "prop": np.ascontiguousarray(props[b, blk * _RPC : (blk + 1) * _RPC]),
            }
        )

    trace = bool(os.environ.get("BASS_ROI_TRACE"))
    kwargs = {}
    if trace:
        kwargs = {"trace": True, "tmpdir": os.environ.get("BASS_ROI_TRACE_DIR") or None}
    res = run_bass_kernel_spmd(nc, in_maps, core_ids=list(range(_NCORES)), **kwargs)
    if trace:
        print(f"HW exec time: {res.exec_time_ns} ns")
        if res.instructions_and_trace:
            print("trace:", res.instructions_and_trace[1])

    out = np.empty((_B, _R, 7, 7, _C), dtype=np.float32)
    for c in range(_NCORES):
        b, blk = divmod(c, _NCORES // _B)
        out[b, blk * _RPC : (blk + 1) * _RPC] = res.results[c]["out"].reshape(_RPC, 7, 7, _C)
    return out
